# revision 29
# baseline (speedup 1.0000x reference)
"""GPS (GIN + global attention) kernel for 8 TRN2 NeuronCores — v2.

Sharding: branch outputs sharded 512 nodes/core; the post-attention tail
(h, MLP, BN3 stats) is computed fully REPLICATED on every core so the only
collectives are two AllGathers per layer:
  AG1: u1 (GIN branch out + BN1 partial stats) — launched before the
       attention core, fully hidden under its exp work.
  AG2: u2 (attention branch out + BN2 partial stats) — semi-hidden by
       AG1 consumption (u1_full DMA, BN1 params, ha).
No BN-stats AllReduces: BN1/BN2 partial sums ride the AG payloads and are
reduced post-gather; BN3 stats are computed locally on the replicated y.
Own-column state (xq) is maintained by recomputing the tail on the own
512 columns (u1_own/u2_own are already resident) — no per-core slicing
of replicated tensors is ever needed.

Kept from v1:
- GIN aggregation = dense (A+I).T slice matmul; adjacency resident in SBUF
  bf16 (+I folds the residual add).
- BN3 stays folded: xT holds RAW y; the next layer applies s3/t3 via the
  aggregation fixup (s3 scale + t3*(1+indeg) rank-1) and s3-scaled QKV
  weights with W@t3 bias corrections.
- Attention scores ST[k, q] per head; denominator via ones column in v_aug.
New in v2:
- exp batched over PAIRS of (h,kt) score tiles: two QK matmuls fill halves
  of one [128,1024] 2-bank PSUM tile, one ACT exp per pair (64 instead of
  128 ACT instructions/layer).
- softmax normalization deferred until after the full QK/exp/AV stream;
  reciprocal computed as Rsqrt(d)^2 and BN rsqrt as direct Rsqrt, so ACT
  uses only {exp, relu, rsqrt}: 2 table-set loads per layer.
"""
import sys
sys.path.insert(0, "/opt/trn_rl_repo")

import numpy as np
import ml_dtypes
import concourse.bacc as bacc
import concourse.mybir as mybir
import concourse.tile as tile
from concourse import bass_utils

N = 4096
C = 128
L = 4
E = 131072
HEADS = 4
D = C // HEADS            # 32
NCORES = 8
NO = N // NCORES          # 512 nodes per core
BN_EPS = 1e-5
RG = [[i for i in range(NCORES)]]

f32 = mybir.dt.float32
f32r = mybir.dt.float32r
f16 = mybir.dt.float16
f16b = mybir.dt.bfloat16
AF = mybir.ActivationFunctionType
OP = mybir.AluOpType
AX = mybir.AxisListType

_CACHED = {}


def _build(depth_mult=1, nocc=False, noexp=False):
    nc = bacc.Bacc("TRN2", target_bir_lowering=False, num_devices=NCORES)

    # ---------------- DRAM I/O ----------------
    d_xT = nc.dram_tensor("xT", [C, N], f32r, kind="ExternalInput")
    d_xq = nc.dram_tensor("xq", [C, NO], f32r, kind="ExternalInput")       # per-core
    d_AT = nc.dram_tensor("AT", [N, NO], f16b, kind="ExternalInput")       # per-core bf16
    d_ident = nc.dram_tensor("ident", [C, C], f32r, kind="ExternalInput")
    d_ones = nc.dram_tensor("onesc", [C, C], f32r, kind="ExternalInput")
    d_wiT = nc.dram_tensor("wiT", [L, C, 3 * C + 64], f32r, kind="ExternalInput")
    d_wib = nc.dram_tensor("wib", [L, C, 5], f32, kind="ExternalInput")
    d_woTh = nc.dram_tensor("woTh", [L, D, 4 * C], f32r, kind="ExternalInput")
    d_wob = nc.dram_tensor("wob", [L, C, 1], f32, kind="ExternalInput")
    d_g1T = nc.dram_tensor("g1T", [L, C, C], f32r, kind="ExternalInput")
    d_g1b = nc.dram_tensor("g1b", [L, C, 1], f32, kind="ExternalInput")
    d_g2T = nc.dram_tensor("g2T", [L, C, C], f32r, kind="ExternalInput")
    d_g2b = nc.dram_tensor("g2b", [L, C, 1], f32, kind="ExternalInput")
    d_m1T = nc.dram_tensor("m1T", [L, C, 2 * C], f16, kind="ExternalInput")
    d_m1b = nc.dram_tensor("m1b", [L, C, 2], f32, kind="ExternalInput")
    d_m2T = nc.dram_tensor("m2T", [L, C, 2, C], f16, kind="ExternalInput")
    d_m2b = nc.dram_tensor("m2b", [L, C, 1], f32, kind="ExternalInput")
    d_bng = nc.dram_tensor("bng", [L, 3, C, 1], f32, kind="ExternalInput")
    d_bnb = nc.dram_tensor("bnb", [L, 3, C, 1], f32, kind="ExternalInput")
    d_hw1 = nc.dram_tensor("hw1T", [C, C // 2], f32r, kind="ExternalInput")
    d_hw2 = nc.dram_tensor("hw2T", [C // 2, C // 4], f32r, kind="ExternalInput")
    d_hw3 = nc.dram_tensor("hw3T", [C // 4, 1], f32r, kind="ExternalInput")
    d_hb1 = nc.dram_tensor("hb1", [C // 2, 1], f32, kind="ExternalInput")
    d_hb2 = nc.dram_tensor("hb2", [C // 4, 1], f32, kind="ExternalInput")
    d_hb3 = nc.dram_tensor("hb3", [1, 1], f32, kind="ExternalInput")
    d_ind = nc.dram_tensor("ind1b", [C, NO], f32r, kind="ExternalInput")   # per-core 1+indeg bcast
    d_out = nc.dram_tensor("out", [1, NO], f32, kind="ExternalOutput")

    KT = N // C               # 32 k-tiles of 128 nodes

    with tile.TileContext(nc) as tc:
        with tc.tile_pool(name="const", bufs=1) as cp, \
             tc.tile_pool(name="big", bufs=1) as bp, \
             tc.tile_pool(name="work", bufs=1) as wp, \
             tc.tile_pool(name="stream", bufs=4) as sp, \
             tc.tile_pool(name="psum_st", bufs=2, space="PSUM") as pst, \
             tc.tile_pool(name="psum_ot", bufs=1, space="PSUM") as pot, \
             tc.tile_pool(name="psum_mm", bufs=2, space="PSUM") as pmm, \
             tc.tile_pool(name="dram", bufs=2, space="DRAM") as dp:

            # ---------------- constants ----------------
            ident = cp.tile([C, C], f32r, name="ident")
            nc.sync.dma_start(out=ident[:], in_=d_ident[:])
            xT = bp.tile([C, N], f32r, name="xT")          # full features (raw y)
            nc.sync.dma_start(out=xT[:], in_=d_xT[:])
            xq = bp.tile([C, NO], f32r, name="xq")         # own columns (raw y)
            nc.sync.dma_start(out=xq[:], in_=d_xq[:])
            onesc = cp.tile([C, C], f32r, name="onesc")
            nc.sync.dma_start(out=onesc[:], in_=d_ones[:])
            epsc = cp.tile([C, 1], f32, name="epsc")
            nc.vector.memset(epsc[:], BN_EPS)
            identh = cp.tile([C, C], f16, name="identh")
            nc.vector.tensor_copy(out=identh[:], in_=ident[:])
            onesh = cp.tile([C, C // 2], f16, name="onesh")
            nc.vector.memset(onesh[:], 1.0)

            W = {}

            def load_layer_weights(i):
                for key, dt_, dram, shp in (
                    ("wiT", f32r, d_wiT, [C, 3 * C + 64]),
                    ("wib", f32, d_wib, [C, 5]),
                    ("woTh", f32r, d_woTh, [D, 4 * C]),
                    ("wob", f32, d_wob, [C, 1]),
                    ("g1T", f32r, d_g1T, [C, C]),
                    ("g1b", f32, d_g1b, [C, 1]),
                    ("g2T", f32r, d_g2T, [C, C]),
                    ("g2b", f32, d_g2b, [C, 1]),
                    ("m1T", f16, d_m1T, [C, 2 * C]),
                    ("m1b", f32, d_m1b, [C, 2]),
                    ("m2T", f16, d_m2T, [C, 2, C]),
                    ("m2b", f32, d_m2b, [C, 1]),
                ):
                    t = cp.tile(shp, dt_, name=f"{key}_{i}")
                    nc.sync.dma_start(out=t[:], in_=dram[i])
                    W[(key, i)] = t
                for j, key in ((0, "n1"), (1, "n2"), (2, "n3")):
                    tg = cp.tile([C, 1], f32, name=f"{key}g_{i}")
                    nc.sync.dma_start(out=tg[:], in_=d_bng[i, j])
                    tb = cp.tile([C, 1], f32, name=f"{key}b_{i}")
                    nc.sync.dma_start(out=tb[:], in_=d_bnb[i, j])
                    W[(key + "g", i)] = tg
                    W[(key + "b", i)] = tb

            load_layer_weights(0)
            hw1 = cp.tile([C, C // 2], f32r, name="hw1")
            nc.sync.dma_start(out=hw1[:], in_=d_hw1[:])
            hw2 = cp.tile([C // 2, C // 4], f32r, name="hw2")
            nc.sync.dma_start(out=hw2[:], in_=d_hw2[:])
            hw3 = cp.tile([C // 4, 1], f32r, name="hw3")
            nc.sync.dma_start(out=hw3[:], in_=d_hw3[:])
            hb1 = cp.tile([C // 2, 1], f32, name="hb1")
            nc.sync.dma_start(out=hb1[:], in_=d_hb1[:])
            hb2 = cp.tile([C // 4, 1], f32, name="hb2")
            nc.sync.dma_start(out=hb2[:], in_=d_hb2[:])
            hb3 = cp.tile([1, 1], f32, name="hb3")
            nc.sync.dma_start(out=hb3[:], in_=d_hb3[:])
            ind1b = cp.tile([C, NO], f32r, name="ind1b")
            nc.sync.dma_start(out=ind1b[:], in_=d_ind[:])

            # ---------------- persistent feature tiles ----------------
            at_res = bp.tile([C, KT, NO], f16b, name="at_res")
            for jc in range(8):
                nc.sync.dma_start(
                    out=at_res[:, 4 * jc:4 * (jc + 1), :],
                    in_=d_AT[4 * jc * C:4 * (jc + 1) * C, :].rearrange(
                        "(j p) c -> p j c", p=C))
            x_nm = bp.tile([C, KT, C], f16b, name="x_nm")  # node-major raw y (bf16)
            v_aug = wp.tile([C, KT, 33 * HEADS], f16, name="v_aug", tag="v_aug")
            u1f = bp.tile([C, N], f16, name="u1f")         # gathered u1; becomes ha
            u2f = bp.tile([C, N], f16, name="u2f")         # gathered u2; becomes hh

            def build_x_nm(li):
                for j in range(KT):
                    tp = pst.tile([C, C], f32r, name=f"tp_{li}_{j}", tag="st")
                    nc.tensor.transpose(tp[:], xT[:, j * C:(j + 1) * C], ident[:])
                    if j % 2 == 0:
                        nc.vector.tensor_copy(out=x_nm[:, j, :], in_=tp[:])
                    else:
                        nc.scalar.activation(x_nm[:, j, :], tp[:], AF.Copy)

            build_x_nm("init")

            def bn_params(s_sum, s_sq, g_ap, b_ap, nm, tb, denom=float(N)):
                """From global [128,1] sum/sumsq -> (s, t) with BN(u) = s*u + t."""
                mean = wp.tile([C, 1], f32, name=f"mean_{nm}", tag=f"bnp_{tb}0")
                nc.vector.tensor_scalar(out=mean[:], in0=s_sum, scalar1=1.0 / denom,
                                        scalar2=None, op0=OP.mult)
                var = wp.tile([C, 1], f32, name=f"var_{nm}", tag=f"bnp_{tb}1")
                nc.vector.tensor_scalar(out=var[:], in0=s_sq, scalar1=1.0 / denom,
                                        scalar2=None, op0=OP.mult)
                msq = wp.tile([C, 1], f32, name=f"msq_{nm}", tag=f"bnp_{tb}2")
                nc.vector.tensor_tensor(out=msq[:], in0=mean[:], in1=mean[:], op=OP.mult)
                nc.vector.tensor_tensor(out=var[:], in0=var[:], in1=msq[:], op=OP.subtract)
                sqv = wp.tile([C, 1], f32, name=f"sqv_{nm}", tag=f"bnp_{tb}3")
                nc.scalar.activation(sqv[:], var[:], AF.Sqrt, bias=epsc[:], scale=1.0)
                rst = wp.tile([C, 1], f32, name=f"rst_{nm}", tag=f"bnp_{tb}4")
                nc.vector.reciprocal_approx_fast(out=rst[:], in_=sqv[:])
                s_ = wp.tile([C, 1], f32, name=f"s_{nm}", tag=f"bnp_{tb}5")
                nc.vector.tensor_tensor(out=s_[:], in0=rst[:], in1=g_ap, op=OP.mult)
                sm = wp.tile([C, 1], f32, name=f"sm_{nm}", tag=f"bnp_{tb}6")
                nc.vector.tensor_tensor(out=sm[:], in0=s_[:], in1=mean[:], op=OP.mult)
                t_ = wp.tile([C, 1], f32, name=f"t_{nm}", tag=f"bnp_{tb}7")
                nc.vector.tensor_tensor(out=t_[:], in0=b_ap, in1=sm[:], op=OP.subtract)
                return s_, t_

            def sumsq_into(u_ap, dst2_ap, nm):
                """Write rowsumsq/NO into dst2[:,1:2] (f16-safe scaling) via an
                ACT Square with fused row accumulation (ACT is idle here)."""
                sq = wp.tile([C, NO], f32, name=f"sq_{nm}", tag="zi")
                ss = wp.tile([C, 1], f32, name=f"ss_{nm}", tag="ssq")
                nc.scalar.activation(sq[:], u_ap, AF.Square, accum_out=ss[:])
                with nc.allow_low_precision(reason="bn partial sums in f16 payload"):
                    nc.vector.tensor_scalar(out=dst2_ap[:, 1:2], in0=ss[:],
                                            scalar1=1.0 / NO, scalar2=None,
                                            op0=OP.mult)

            def allgather(payload, nm, ring):
                ag_in = dp.tile([C, NO + 4], f16, name=f"agi_{nm}", tag=f"agi{ring}")
                nc.sync.dma_start(out=ag_in[:], in_=payload[:])
                ag_out = dp.tile([NCORES, C, NO + 4], f16, name=f"ago_{nm}",
                                 tag=f"ago{ring}",
                                 addr_space="Local" if nocc else "Shared")
                if nocc:
                    for _r in range(NCORES):
                        nc.sync.dma_start(out=ag_out[_r], in_=ag_in[:])
                else:
                    nc.gpsimd.collective_compute(
                        "AllGather", OP.bypass, replica_groups=RG,
                        ins=[ag_in[:].opt()], outs=[ag_out[:].opt()])
                return ag_out

            def consume_ag(ag_out, full_tile, g_ap, b_ap, nm, tb):
                """DMA gathered payload into [C, N]; reduce stats; bn params."""
                nc.sync.dma_start(
                    out=full_tile[:].rearrange("p (r c) -> p r c", r=NCORES),
                    in_=ag_out[:, :, 0:NO].rearrange("r p c -> p r c"))
                stp = wp.tile([C, NCORES, 2], f16, name=f"stp_{nm}", tag=f"stp_{tb}")
                nc.sync.dma_start(out=stp[:],
                                  in_=ag_out[:, :, NO:NO + 2].rearrange("r p c -> p r c"))
                stg = wp.tile([C, 2], f32, name=f"stg_{nm}", tag=f"stg_{tb}")
                nc.vector.reduce_sum(stg[:],
                                     stp[:].rearrange("p r c -> p c r"), axis=AX.X)
                return bn_params(stg[:, 0:1], stg[:, 1:2], g_ap, b_ap, nm, tb,
                                 denom=float(NCORES))

            s3p = t3p = None   # BN3 params pending application (folded)
            for li in range(L * depth_mult):
                i = li % L
                is_last = li == L * depth_mult - 1

                # ---- own x with BN3 applied (residual base) ----
                if s3p is None:
                    xo = xq
                else:
                    xo = wp.tile([C, NO], f32r, name=f"xo_{li}", tag="xo")
                    nc.vector.tensor_scalar(out=xo[:], in0=xq[:], scalar1=s3p[:],
                                            scalar2=t3p[:], op0=OP.mult, op1=OP.add)

                # ---- fold s3 into attention input weights; biases get W @ t3 ----
                if s3p is None:
                    wi_ap = W[("wiT", i)]
                    bq, bk, bv = (W[("wib", i)][:, 0:1], W[("wib", i)][:, 1:2],
                                  W[("wib", i)][:, 2:3])
                    bk3, bq3 = W[("wib", i)][0:D, 3:4], W[("wib", i)][0:D, 4:5]
                else:
                    wi_ap = wp.tile([C, 3 * C + 64], f32r, name=f"wiTs_{li}", tag="wiTs")
                    nc.vector.tensor_scalar(out=wi_ap[:], in0=W[("wiT", i)][:],
                                            scalar1=s3p[:], scalar2=None, op0=OP.mult)
                    t3r = wp.tile([C, 8], f32r, name=f"t3r_{li}", tag="t3r")
                    nc.vector.tensor_copy(out=t3r[:], in_=t3p[:].to_broadcast([C, 8]))
                    eb = wp.tile([C, 5], f32, name=f"eb_{li}", tag="eb")
                    for bi_, (c0, c1, w_) in enumerate(((0, C, C), (C, 2 * C, C),
                                                        (2 * C, 3 * C, C),
                                                        (3 * C, 3 * C + D, D),
                                                        (3 * C + D, 3 * C + 2 * D, D))):
                        ebp = pmm.tile([w_, 8], f32, name=f"ebp_{li}_{bi_}", tag="mm")
                        nc.tensor.matmul(ebp[:], W[("wiT", i)][:, c0:c1], t3r[:],
                                         start=True, stop=True)
                        nc.vector.tensor_tensor(out=eb[0:w_, bi_:bi_ + 1], in0=ebp[:, 0:1],
                                                in1=W[("wib", i)][0:w_, bi_:bi_ + 1],
                                                op=OP.add)
                    bq, bk, bv = eb[:, 0:1], eb[:, 1:2], eb[:, 2:3]
                    bk3, bq3 = eb[0:D, 3:4], eb[0:D, 4:5]

                # ---- QKV projections (K/V/K3 on full xT; Q on own xq) ----
                qp = pmm.tile([C, NO], f32, name=f"qp_{li}", tag="mm")
                nc.tensor.matmul(qp[:], wi_ap[:, 0:C], xq[:], start=True, stop=True)
                qTs = wp.tile([C, NO], f16, name=f"qTs_{li}", tag="qTs")
                nc.vector.tensor_scalar(out=qTs[:], in0=qp[:], scalar1=bq,
                                        scalar2=None, op0=OP.add)
                qp3 = pmm.tile([D, NO], f32, name=f"qp3_{li}", tag="mm")
                nc.tensor.matmul(qp3[:], wi_ap[:, 3 * C + 32:3 * C + 64], xq[:],
                                 start=True, stop=True)
                qTs3 = wp.tile([D, NO], f16, name=f"qTs3_{li}", tag="qTs3")
                nc.vector.tensor_scalar(out=qTs3[:], in0=qp3[:], scalar1=bq3,
                                        scalar2=None, op0=OP.add)
                kTs3 = wp.tile([D, N], f16, name=f"kTs3_{li}", tag="kTs3")
                kTs = wp.tile([C, N], f16, name=f"kTs_{li}", tag="kTs")
                vTs = wp.tile([C, N], f16, name=f"vTs_{li}", tag="vTs")
                for cch in range(NCORES):
                    csl = slice(cch * 512, (cch + 1) * 512)
                    kp = pmm.tile([C, 512], f32, name=f"kp_{li}_{cch}", tag="mm")
                    nc.tensor.matmul(kp[:], wi_ap[:, C:2 * C], xT[:, csl],
                                     start=True, stop=True)
                    nc.vector.tensor_scalar(out=kTs[:, csl], in0=kp[:], scalar1=bk,
                                            scalar2=None, op0=OP.add)
                    vp = pmm.tile([C, 512], f32, name=f"vp_{li}_{cch}", tag="mm")
                    nc.tensor.matmul(vp[:], wi_ap[:, 2 * C:3 * C], xT[:, csl],
                                     start=True, stop=True)
                    nc.vector.tensor_scalar(out=vTs[:, csl], in0=vp[:], scalar1=bv,
                                            scalar2=None, op0=OP.add)
                    kp3 = pmm.tile([D, 512], f32, name=f"kp3_{li}_{cch}", tag="mm")
                    nc.tensor.matmul(kp3[:], wi_ap[:, 3 * C:3 * C + 32], xT[:, csl],
                                     start=True, stop=True)
                    nc.vector.tensor_scalar(out=kTs3[:, csl], in0=kp3[:], scalar1=bk3,
                                            scalar2=None, op0=OP.add)
                # v -> node-major into v_aug (ones column last per head)
                nc.vector.tensor_copy(
                    out=v_aug[:, :, :].rearrange("p kt (h c) -> p kt h c", h=HEADS)[:, :, :, 32:33],
                    in_=onesc[:, 0:1].to_broadcast([C, KT, HEADS, 1]))
                for kt in range(KT):
                    vt = pst.tile([C, C], f16, name=f"vt_{li}_{kt}", tag="st")
                    nc.tensor.transpose(vt[:], vTs[:, kt * C:(kt + 1) * C], identh[:])
                    nc.vector.tensor_copy(
                        out=v_aug[:, kt, :].rearrange("p (h c) -> p h c",
                                                      h=HEADS)[:, :, 0:32],
                        in_=vt[:].rearrange("p (h c) -> p h c", h=HEADS))

                # ---- GIN branch -> u1_own in payload1; AG1 launch ----
                z = wp.tile([C, NO], f32r, name=f"z_{li}", tag="z")
                agg = pmm.tile([C, NO], f32, name=f"agg_{li}", tag="apmm", bufs=1)
                for j in range(KT):
                    nc.tensor.matmul(agg[:], x_nm[:, j, :], at_res[:, j, :],
                                     start=(j == 0), stop=(j == KT - 1))
                if s3p is None:
                    nc.vector.tensor_copy(out=z[:], in_=agg[:])
                else:
                    nc.vector.tensor_scalar(out=z[:], in0=agg[:], scalar1=s3p[:],
                                            scalar2=None, op0=OP.mult)
                    zi = wp.tile([C, NO], f32, name=f"zi_{li}", tag="zi")
                    nc.vector.tensor_scalar(out=zi[:], in0=ind1b[:], scalar1=t3p[:],
                                            scalar2=None, op0=OP.mult)
                    nc.vector.tensor_tensor(out=z[:], in0=z[:], in1=zi[:], op=OP.add)
                if li == 0:
                    for ii in range(1, L):
                        load_layer_weights(ii)
                g1p = pmm.tile([C, NO], f32, name=f"g1p_{li}", tag="mm")
                nc.tensor.matmul(g1p[:], W[("g1T", i)][:], z[:], start=True, stop=True)
                r1 = wp.tile([C, NO], f32r, name=f"r1_{li}", tag="r1")
                nc.scalar.activation(r1[:], g1p[:], AF.Relu, bias=W[("g1b", i)][:], scale=1.0)
                g2p = pmm.tile([C, NO], f32, name=f"g2p_{li}", tag="mm")
                nc.tensor.matmul(g2p[:], W[("g2T", i)][:], r1[:], start=True, stop=True)
                pay1 = wp.tile([C, NO + 4], f16, name=f"pay1_{li}", tag="pay1")
                nc.vector.tensor_copy(out=pay1[:, NO + 2:NO + 4],
                                       in_=epsc[:].to_broadcast([C, 2]))
                u1q = pay1[:, 0:NO]
                s1sum = wp.tile([C, 1], f32, name=f"s1sum_{li}", tag="s1sum")
                nc.vector.scalar_tensor_tensor(out=u1q, in0=g2p[:],
                                               scalar=W[("g2b", i)][:], in1=xo[:],
                                               op0=OP.add, op1=OP.add,
                                               accum_out=s1sum[:])
                with nc.allow_low_precision(reason="bn partial sums in f16 payload"):
                    nc.vector.tensor_scalar(out=pay1[:, NO:NO + 1], in0=s1sum[:],
                                            scalar1=1.0 / NO, scalar2=None,
                                            op0=OP.mult)
                sumsq_into(u1q, pay1[:, NO:NO + 2], f"u1_{li}")
                ag1 = allgather(pay1, f"u1_{li}", "1")

                # ---- attention core: exp batched over pairs, deferred norm ----
                ap_ = pmm.tile([C, NO], f32, name=f"ap_{li}", tag="apmm", bufs=1)
                ot_sb = wp.tile([33, HEADS * 512], f16, name=f"otsb_{li}", tag="otsb")
                ots = [None] * HEADS
                pairs = [(h, kt) for h in range(HEADS) for kt in range(KT)]
                NG = len(pairs) // 2
                ests = {}
                LAG = 2

                def qk_grp(g):
                    st = pst.tile([C, 2 * NO], f32, name=f"st_{li}_{g}", tag="st")
                    for s_ in (0, 1):
                        h, kt = pairs[2 * g + s_]
                        osl = slice(s_ * NO, (s_ + 1) * NO)
                        if h < 3:
                            hsl = slice(h * D, (h + 1) * D)
                            nc.tensor.matmul(st[:, osl], kTs[hsl, kt * C:(kt + 1) * C],
                                             qTs[hsl, :], start=True, stop=True)
                        else:
                            nc.tensor.matmul(st[:, osl], kTs3[:, kt * C:(kt + 1) * C],
                                             qTs3[:, :], start=True, stop=True)
                    est = sp.tile([C, 2 * NO], f16, name=f"est_{li}_{g}", tag="est",
                                  bufs=3)
                    if noexp:
                        nc.vector.tensor_copy(out=est[:], in_=st[:])
                    else:
                        nc.scalar.activation(est[:], st[:], AF.Exp)
                    ests[g] = est

                def av_grp(g):
                    est = ests.pop(g)
                    for s_ in (0, 1):
                        h, kt = pairs[2 * g + s_]
                        if kt == 0:
                            ots[h] = pot.tile([33, 512], f32, name=f"ot_{li}_{h}",
                                              tag="ot")
                        nc.tensor.matmul(ots[h][:], v_aug[:, kt, 33 * h:33 * (h + 1)],
                                         est[:, s_ * NO:(s_ + 1) * NO],
                                         start=(kt == 0), stop=(kt == KT - 1))
                        if kt == KT - 1:
                            nc.vector.tensor_copy(
                                out=ot_sb[:, h * 512:(h + 1) * 512], in_=ots[h][:])
                            # inline normalization + out-projection for this
                            # head; overlaps the remaining heads' QK/exp/AV
                            rbp = pmm.tile([32, 512], f32, name=f"rbp_{li}_{h}",
                                           tag="mm")
                            nc.tensor.matmul(rbp[:], onesh[32:33, 0:32],
                                             ot_sb[32:33, h * 512:(h + 1) * 512],
                                             start=True, stop=True)
                            recd = wp.tile([32, 512], f32, name=f"recd_{li}_{h}",
                                           tag="recd")
                            nc.vector.reciprocal_approx_fast(out=recd[:], in_=rbp[:])
                            on = wp.tile([32, 512], f32r, name=f"on_{li}_{h}",
                                         tag="on", bufs=2)
                            nc.vector.tensor_tensor(
                                out=on[:],
                                in0=ot_sb[0:32, h * 512:(h + 1) * 512],
                                in1=recd[:], op=OP.mult)
                            nc.tensor.matmul(ap_[:],
                                             W[("woTh", i)][:, h * C:(h + 1) * C],
                                             on[:], start=(h == 0),
                                             stop=(h == HEADS - 1))

                for g in range(NG):
                    qk_grp(g)
                    if g >= LAG:
                        av_grp(g - LAG)
                for g in range(NG - LAG, NG):
                    av_grp(g)

                pay2 = wp.tile([C, NO + 4], f16, name=f"pay2_{li}", tag="pay2")
                nc.vector.tensor_copy(out=pay2[:, NO + 2:NO + 4],
                                       in_=epsc[:].to_broadcast([C, 2]))
                u2q = pay2[:, 0:NO]
                s2sum = wp.tile([C, 1], f32, name=f"s2sum_{li}", tag="s2sum")
                nc.vector.scalar_tensor_tensor(out=u2q, in0=ap_[:],
                                               scalar=W[("wob", i)][:], in1=xo[:],
                                               op0=OP.add, op1=OP.add,
                                               accum_out=s2sum[:])
                with nc.allow_low_precision(reason="bn partial sums in f16 payload"):
                    nc.vector.tensor_scalar(out=pay2[:, NO:NO + 1], in0=s2sum[:],
                                            scalar1=1.0 / NO, scalar2=None,
                                            op0=OP.mult)
                sumsq_into(u2q, pay2[:, NO:NO + 2], f"u2_{li}")
                ag2 = allgather(pay2, f"u2_{li}", "2")

                # ---- consume AG1 (hidden under AG2): u1f -> ha in place ----
                s1, t1 = consume_ag(ag1, u1f, W[("n1g", i)][:], W[("n1b", i)][:],
                                    f"bn1_{li}", "b1")
                nc.vector.tensor_scalar(out=u1f[:], in0=u1f[:], scalar1=s1[:],
                                        scalar2=t1[:], op0=OP.mult, op1=OP.add)
                # ---- consume AG2: u2f -> hb -> hh in place ----
                s2, t2 = consume_ag(ag2, u2f, W[("n2g", i)][:], W[("n2b", i)][:],
                                    f"bn2_{li}", "b2")
                nc.vector.tensor_scalar(out=u2f[:], in0=u2f[:], scalar1=s2[:],
                                        scalar2=t2[:], op0=OP.mult, op1=OP.add)
                nc.vector.tensor_tensor(out=u2f[:], in0=u2f[:], in1=u1f[:], op=OP.add)

                # ---- own-column tail (keeps xq without per-core slicing) ----
                hq = wp.tile([C, NO], f16, name=f"hq_{li}", tag="hq")
                nc.vector.tensor_scalar(out=hq[:], in0=u1q, scalar1=s1[:],
                                        scalar2=t1[:], op0=OP.mult, op1=OP.add)
                hbq = wp.tile([C, NO], f32, name=f"hbq_{li}", tag="zi")
                nc.vector.tensor_scalar(out=hbq[:], in0=u2q, scalar1=s2[:],
                                        scalar2=t2[:], op0=OP.mult, op1=OP.add)
                nc.vector.tensor_tensor(out=hq[:], in0=hq[:], in1=hbq[:], op=OP.add)
                m1pq = pmm.tile([C, NO], f32, name=f"m1pq_{li}", tag="mm")
                nc.tensor.matmul(m1pq[:], W[("m1T", i)][:, 0:C], hq[:],
                                 start=True, stop=True)
                raq = wp.tile([C, NO], f16, name=f"raq_{li}", tag="ra", bufs=2)
                nc.scalar.activation(raq[:], m1pq[:], AF.Relu,
                                     bias=W[("m1b", i)][:, 0:1], scale=1.0)
                m1pq2 = pmm.tile([C, NO], f32, name=f"m1pq2_{li}", tag="mm")
                nc.tensor.matmul(m1pq2[:], W[("m1T", i)][:, C:2 * C], hq[:],
                                 start=True, stop=True)
                rbq = wp.tile([C, NO], f16, name=f"rbq_{li}", tag="rb", bufs=2)
                nc.scalar.activation(rbq[:], m1pq2[:], AF.Relu,
                                     bias=W[("m1b", i)][:, 1:2], scale=1.0)
                m2pq = pmm.tile([C, NO], f32, name=f"m2pq_{li}", tag="mm")
                nc.tensor.matmul(m2pq[:], W[("m2T", i)][:, 0, :], raq[:],
                                 start=True, stop=False)
                nc.tensor.matmul(m2pq[:], W[("m2T", i)][:, 1, :], rbq[:],
                                 start=False, stop=True)
                yq = wp.tile([C, NO], f32r, name=f"yq_{li}", tag="yq")
                nc.vector.tensor_scalar(out=yq[:], in0=m2pq[:], scalar1=W[("m2b", i)][:],
                                        scalar2=None, op0=OP.add)
                nc.vector.tensor_tensor(out=yq[:], in0=yq[:], in1=hq[:], op=OP.add)

                # ---- replicated full tail: y = hh + MLP(hh) into xT ----
                st3p = wp.tile([C, NCORES, 2], f32, name=f"st3p_{li}", tag="st3p")
                for ch in range(NCORES):
                    csl = slice(ch * 512, (ch + 1) * 512)
                    m1a = pmm.tile([C, 512], f32, name=f"m1a_{li}_{ch}", tag="mm")
                    nc.tensor.matmul(m1a[:], W[("m1T", i)][:, 0:C], u2f[:, csl],
                                     start=True, stop=True)
                    ra = wp.tile([C, 512], f16, name=f"ra_{li}_{ch}", tag="ra", bufs=2)
                    nc.scalar.activation(ra[:], m1a[:], AF.Relu,
                                         bias=W[("m1b", i)][:, 0:1], scale=1.0)
                    m1b_ = pmm.tile([C, 512], f32, name=f"m1b_{li}_{ch}", tag="mm")
                    nc.tensor.matmul(m1b_[:], W[("m1T", i)][:, C:2 * C], u2f[:, csl],
                                     start=True, stop=True)
                    rb_ = wp.tile([C, 512], f16, name=f"rb_{li}_{ch}", tag="rb", bufs=2)
                    nc.vector.tensor_scalar(out=rb_[:], in0=m1b_[:],
                                            scalar1=W[("m1b", i)][:, 1:2],
                                            scalar2=0.0, op0=OP.add, op1=OP.max)
                    m2p = pmm.tile([C, 512], f32, name=f"m2p_{li}_{ch}", tag="mm")
                    nc.tensor.matmul(m2p[:], W[("m2T", i)][:, 0, :], ra[:],
                                     start=True, stop=False)
                    nc.tensor.matmul(m2p[:], W[("m2T", i)][:, 1, :], rb_[:],
                                     start=False, stop=True)
                    nc.vector.scalar_tensor_tensor(out=xT[:, csl], in0=m2p[:],
                                                   scalar=W[("m2b", i)][:],
                                                   in1=u2f[:, csl],
                                                   op0=OP.add, op1=OP.add,
                                                   accum_out=st3p[:, ch, 0:1])
                    sqc = wp.tile([C, 512], f32, name=f"sqc_{li}_{ch}", tag="zi")
                    nc.scalar.activation(sqc[:], xT[:, csl], AF.Square,
                                         accum_out=st3p[:, ch, 1:2])

                # ---- BN3 stats: reduce the 8 chunk partials ----
                st3 = wp.tile([C, 2], f32, name=f"st3_{li}", tag="st3")
                nc.vector.reduce_sum(st3[:],
                                     st3p[:].rearrange("p r c -> p c r"), axis=AX.X)
                s3, t3 = bn_params(st3[:, 0:1], st3[:, 1:2],
                                   W[("n3g", i)][:], W[("n3b", i)][:], f"bn3_{li}", "b3")

                if not is_last:
                    nc.vector.tensor_copy(out=xq[:], in_=yq[:])
                    build_x_nm(li)
                    s3p, t3p = s3, t3
                else:
                    xf = wp.tile([C, NO], f32r, name="xf", tag="xf")
                    nc.vector.tensor_scalar(out=xf[:], in0=yq[:], scalar1=s3[:],
                                            scalar2=t3[:], op0=OP.mult, op1=OP.add)
                    h1p = pmm.tile([C // 2, NO], f32, name="h1p", tag="mm")
                    nc.tensor.matmul(h1p[:], hw1[:], xf[:], start=True, stop=True)
                    hr1 = wp.tile([C // 2, NO], f32r, name="hr1", tag="hr1")
                    nc.scalar.activation(hr1[:], h1p[:], AF.Relu, bias=hb1[:], scale=1.0)
                    h2p = pmm.tile([C // 4, NO], f32, name="h2p", tag="mm")
                    nc.tensor.matmul(h2p[:], hw2[:], hr1[:], start=True, stop=True)
                    hr2 = wp.tile([C // 4, NO], f32r, name="hr2", tag="hr2")
                    nc.scalar.activation(hr2[:], h2p[:], AF.Relu, bias=hb2[:], scale=1.0)
                    h3p = pmm.tile([1, NO], f32, name="h3p", tag="mm")
                    nc.tensor.matmul(h3p[:], hw3[:], hr2[:], start=True, stop=True)
                    outs = wp.tile([1, NO], f32, name="outs", tag="outs")
                    nc.vector.tensor_scalar(out=outs[:], in0=h3p[:], scalar1=hb3[:],
                                            scalar2=None, op0=OP.add)
                    nc.sync.dma_start(out=d_out[:], in_=outs[:])

    nc.compile()
    return nc


def _host_prep(inputs):
    x = np.asarray(inputs["x"], dtype=np.float32)
    ei = np.asarray(inputs["edge_index"])
    src, dst = np.asarray(ei[0], dtype=np.int64), np.asarray(ei[1], dtype=np.int64)
    AT = np.zeros((N, N), dtype=np.float32)
    np.add.at(AT, (src, dst), 1.0)
    AT[np.arange(N), np.arange(N)] += 1.0      # fold +x into the aggregation
    ind1 = AT.sum(axis=0)                      # 1 + in-degree per node

    xT = np.ascontiguousarray(x.T)
    sd = 1.0 / np.sqrt(np.float32(D))

    common = {
        "xT": xT,
        "ident": np.eye(C, dtype=np.float32),
        "onesc": np.ones((C, C), dtype=np.float32),
        "hw1T": np.ascontiguousarray(np.asarray(inputs["head_w1"], np.float32).T),
        "hw2T": np.ascontiguousarray(np.asarray(inputs["head_w2"], np.float32).T),
        "hw3T": np.ascontiguousarray(np.asarray(inputs["head_w3"], np.float32).T),
        "hb1": np.asarray(inputs["head_b1"], np.float32).reshape(-1, 1),
        "hb2": np.asarray(inputs["head_b2"], np.float32).reshape(-1, 1),
        "hb3": np.asarray(inputs["head_b3"], np.float32).reshape(-1, 1),
    }
    wiT = np.stack([np.asarray(inputs["attn_in_w"][i], np.float32).T for i in range(L)])
    wib = np.stack([np.ascontiguousarray(
        np.asarray(inputs["attn_in_b"][i], np.float32).reshape(3, C).T)
        for i in range(L)])
    wiT = wiT.copy()
    wib = wib.copy()
    wiT[:, :, 0:C] *= sd          # fold 1/sqrt(d) into q projection
    wib[:, :, 0] *= sd
    # head-3 q/k duplicates at partition base 0 (PE quadrant-3 workaround)
    wiT = np.concatenate([wiT,
                          wiT[:, :, C + 3 * D:C + 4 * D],        # k head3
                          wiT[:, :, 3 * D:4 * D]], axis=2)        # q head3 (scaled)
    wib3 = np.zeros((L, C, 2), dtype=np.float32)
    wib3[:, 0:D, 0] = wib[:, 3 * D:4 * D, 1]                      # k head3 bias
    wib3[:, 0:D, 1] = wib[:, 3 * D:4 * D, 0]                      # q head3 bias (scaled)
    wib = np.concatenate([wib, wib3], axis=2)
    woTh = np.zeros((L, D, 4 * C), dtype=np.float32)
    for i in range(L):
        woT = np.asarray(inputs["attn_out_w"][i], np.float32).T   # [C, C]
        for h in range(HEADS):
            woTh[i, :, h * C:(h + 1) * C] = woT[h * D:(h + 1) * D, :]
    common.update({
        "wiT": wiT, "wib": wib, "woTh": woTh,
        "wob": np.stack([np.asarray(inputs["attn_out_b"][i], np.float32).reshape(-1, 1)
                         for i in range(L)]),
        "g1T": np.stack([np.asarray(inputs["gin_w1"][i], np.float32).T for i in range(L)]),
        "g1b": np.stack([np.asarray(inputs["gin_b1"][i], np.float32).reshape(-1, 1)
                         for i in range(L)]),
        "g2T": np.stack([np.asarray(inputs["gin_w2"][i], np.float32).T for i in range(L)]),
        "g2b": np.stack([np.asarray(inputs["gin_b2"][i], np.float32).reshape(-1, 1)
                         for i in range(L)]),
        "m1T": np.stack([np.asarray(inputs["mlp_w1"][i], np.float32).T
                         for i in range(L)]).astype(np.float16),
        "m1b": np.stack([np.ascontiguousarray(
            np.asarray(inputs["mlp_b1"][i], np.float32).reshape(2, C).T)
            for i in range(L)]),
        "m2T": np.stack([np.ascontiguousarray(
            np.asarray(inputs["mlp_w2"][i], np.float32).T.reshape(2, C, C).transpose(1, 0, 2))
            for i in range(L)]).astype(np.float16),
        "m2b": np.stack([np.asarray(inputs["mlp_b2"][i], np.float32).reshape(-1, 1)
                         for i in range(L)]),
        "bng": np.stack([np.stack([np.asarray(inputs[k][i], np.float32).reshape(-1, 1)
                                   for k in ("n1_g", "n2_g", "n3_g")]) for i in range(L)]),
        "bnb": np.stack([np.stack([np.asarray(inputs[k][i], np.float32).reshape(-1, 1)
                                   for k in ("n1_b", "n2_b", "n3_b")]) for i in range(L)]),
    })
    in_maps = []
    for r in range(NCORES):
        m = dict(common)
        m["xq"] = np.ascontiguousarray(xT[:, r * NO:(r + 1) * NO])
        m["AT"] = np.ascontiguousarray(AT[:, r * NO:(r + 1) * NO]).astype(ml_dtypes.bfloat16)
        m["ind1b"] = np.ascontiguousarray(
            np.repeat(ind1[None, r * NO:(r + 1) * NO], C, axis=0)).astype(np.float32)
        in_maps.append(m)
    return in_maps


def kernel(**inputs):
    if "nc" not in _CACHED:
        _CACHED["nc"] = _build()
    nc = _CACHED["nc"]
    in_maps = _host_prep(inputs)
    res = bass_utils.run_bass_kernel_spmd(nc, in_maps, core_ids=list(range(NCORES)))
    y = np.zeros((N, 1), dtype=np.float32)
    for r in range(NCORES):
        y[r * NO:(r + 1) * NO, 0] = res.results[r]["out"][0]
    return y
